# Initial kernel scaffold
#
"""Trainium2 Bass kernel for NeuralBlochRK4.

Reference computation: RK4 integration (255 steps) of dy/dt = MLP([y,u(t),p,t])
with MLP 13 -> 128(tanh) -> 128(tanh) -> 3, batch 16384, output = full
trajectory (B, 256, 3).

Strategy (pure data-parallel over batch, 8 cores x 2048 rows):
  * All elementwise adds are folded into PSUM matmul accumulation, the ACT
    engine's free affine (out = tanh(in + bias)), or one DVE add per step.
  * Per RK4 stage s, z1 = Wc_s^T @ x (K=17 matmul over packed input tile
    [y(3); ones(1); p(5); u_n(4); u_{n+1}(4)]) + alpha_s*(W1_y @ W3) @ h2_{s-1}
    (K=128 matmul, replaces materializing the intermediate y) accumulated in
    PSUM; tanh on ACT with per-step bias w_t*t_n. z2 = W2 @ h1; tanh, bias b2.
  * y_{n+1}: four gamma_s*W3 @ h2_s matmuls accumulate into a (3, W) PSUM
    group; one DVE tensor_add folds in y_n (exact fp32) and writes the next
    x tile; b3 != 0 adds one tensor_scalar op (b3 is zero here).
  * Matmuls run in float32r (single-pass PE mode, ~2x fp32 speed, ~1e-3
    matmul rel err; end-to-end trajectory err ~4e-4 abs, validated vs the
    fp32 reference in numpy).  PSUM accumulation stays fp32.
  * Batch split into 2 interleaved "threads" of 1024 per core so ACT/PE
    pipeline across threads; mms emitted weight-grouped to minimize LDW
    thrash.
  * u pre-transposed on host to (T*4, B_core) so per-step (8, W) DMA slices
    are contiguous.
"""

import numpy as np
from contextlib import ExitStack

import concourse.bass as bass
import concourse.tile as tile
from concourse import bacc, mybir
from concourse.bass_utils import run_bass_kernel_spmd

F32 = mybir.dt.float32
F32R = mybir.dt.float32r
TANH = mybir.ActivationFunctionType.Tanh

B_FULL, T_FULL, HID = 16384, 256, 128
N_CORES = 8


# ----------------------------------------------------------------------------
# host-side constant preparation
# ----------------------------------------------------------------------------

def prepare_consts(W1, b1, W2, b2, W3, b3, t):
    f32 = np.float32
    W1 = np.asarray(W1, f32); W2 = np.asarray(W2, f32); W3 = np.asarray(W3, f32)
    b1 = np.asarray(b1, f32); b2 = np.asarray(b2, f32); b3 = np.asarray(b3, f32)
    t = np.asarray(t, f32)
    h = f32(t[1] - t[0])

    A = W1[:, 0:3]
    U = W1[:, 3:7]
    P = W1[:, 7:12]
    w_t = W1[:, 12]
    C = (A @ W3).astype(f32)
    Ab3 = (A @ b3).astype(f32)

    stages = [
        (f32(0.0), f32(0.0), f32(1.0), f32(0.0)),
        (f32(h / 2), f32(h / 2), f32(0.5), f32(0.5)),
        (f32(h / 2), f32(h / 2), f32(0.5), f32(0.5)),
        (f32(h), f32(h), f32(0.0), f32(1.0)),
    ]
    Wc = []
    for (o, al, cn, ce) in stages:
        kxm = np.zeros((17, 128), f32)
        kxm[0:3, :] = A.T
        kxm[3, :] = b1 + w_t * o + al * Ab3
        kxm[4:9, :] = P.T
        kxm[9:13, :] = cn * U.T
        kxm[13:17, :] = ce * U.T
        Wc.append(np.ascontiguousarray(kxm))

    consts = {
        "Wc1": Wc[0], "Wc23": Wc[1], "Wc4": Wc[3],
        "Ch": np.ascontiguousarray((f32(h / 2) * C.T).astype(f32)),
        "Cf": np.ascontiguousarray((f32(h) * C.T).astype(f32)),
        "W2T": np.ascontiguousarray(W2.T.astype(f32)),
        "W36": np.ascontiguousarray((f32(h / 6) * W3.T).astype(f32)),
        "W33": np.ascontiguousarray((f32(h / 3) * W3.T).astype(f32)),
        "wtt": np.ascontiguousarray(np.outer(w_t, t).astype(f32)),
        "b2": np.ascontiguousarray(b2.reshape(128, 1)),
        "hb3": np.ascontiguousarray((h * b3).reshape(3, 1)),
    }
    return consts


# ----------------------------------------------------------------------------
# device program
# ----------------------------------------------------------------------------

def build_tile_body(tc, aps, B_core, T, NTH, has_b3):
    nc = tc.nc
    W = B_core // NTH          # per-thread batch width
    CH = min(512, W)           # matmul free-dim chunk (one PSUM bank)
    NCH = W // CH
    assert W % CH == 0 and B_core % NTH == 0

    with ExitStack() as ctx:
        wpool = ctx.enter_context(tc.tile_pool(name="wts", bufs=1))
        xpool = ctx.enter_context(tc.tile_pool(name="x", bufs=1))
        h1pool = ctx.enter_context(tc.tile_pool(name="h1", bufs=2))
        h2pool = ctx.enter_context(tc.tile_pool(name="h2", bufs=3))
        zpool = ctx.enter_context(
            tc.tile_pool(name="z", bufs=2, space=bass.MemorySpace.PSUM))
        ypool = ctx.enter_context(
            tc.tile_pool(name="yp", bufs=2, space=bass.MemorySpace.PSUM))

        def wtile(name, shape, dt):
            tl = wpool.tile(list(shape), dt, tag=name)
            nc.sync.dma_start(tl[:, :], aps[name][:, :])
            return tl

        wc1 = wtile("Wc1", (17, 128), F32R)
        wc23 = wtile("Wc23", (17, 128), F32R)
        wc4 = wtile("Wc4", (17, 128), F32R)
        ch_t = wtile("Ch", (128, 128), F32R)
        cf_t = wtile("Cf", (128, 128), F32R)
        w2t = wtile("W2T", (128, 128), F32R)
        w36 = wtile("W36", (128, 3), F32R)
        w33 = wtile("W33", (128, 3), F32R)
        wtt = wtile("wtt", (128, T), F32)
        b2t = wtile("b2", (128, 1), F32)
        hb3t = wtile("hb3", (3, 1), F32) if has_b3 else None

        wc_s = (wc1, wc23, wc23, wc4)
        cs_s = (None, ch_t, ch_t, cf_t)
        w3_s = (w36, w33, w33, w36)

        yout = aps["yout"]      # (3, T-1, B_core) f32r
        uT = aps["uT"]          # (T*4, B_core)   f32r
        xinit = aps["xinit"]    # (17, B_core)    f32r

        # persistent x tiles: [thread][parity]
        xb = []
        for th in range(NTH):
            bufs = []
            for par in range(2):
                tl = xpool.tile([17, W], F32R, tag=f"xb{th}{par}")
                nc.sync.dma_start(tl[:, :], xinit[:, th * W:(th + 1) * W])
                bufs.append(tl)
            xb.append(bufs)
        for th in range(NTH):
            nc.sync.dma_start(xb[th][0][9:17, :], uT[0:8, th * W:(th + 1) * W])
            if T - 1 > 1:
                nc.sync.dma_start(xb[th][1][9:17, :], uT[4:12, th * W:(th + 1) * W])

        csl = [slice(c * CH, (c + 1) * CH) for c in range(NCH)]

        for n in range(T - 1):
            par, nxt = n % 2, (n + 1) % 2

            if n + 1 <= T - 2:
                r0 = 4 * (n + 1)
                for th in range(NTH):
                    nc.sync.dma_start(xb[th][nxt][9:17, :],
                                      uT[r0:r0 + 8, th * W:(th + 1) * W])

            ypsum = [ypool.tile([3, W], F32, tag="yp", name=f"yp{th}")
                     for th in range(NTH)]

            h2prev = [None] * NTH
            for s in range(4):
                # z1 accumulation, weight-grouped: all K17 mms, then all C mms
                z1s = [zpool.tile([128, W], F32, tag="z", name=f"z1_{th}")
                       for th in range(NTH)]
                for th in range(NTH):
                    for sl in csl:
                        nc.tensor.matmul(z1s[th][:, sl], wc_s[s][:, :],
                                         xb[th][par][:, sl],
                                         start=True, stop=(s == 0))
                if s > 0:
                    for th in range(NTH):
                        for sl in csl:
                            nc.tensor.matmul(z1s[th][:, sl], cs_s[s][:, :],
                                             h2prev[th][:, sl],
                                             start=False, stop=True)
                h1s = []
                for th in range(NTH):
                    h1 = h1pool.tile([128, W], F32R, tag="h1", name=f"h1_{th}")
                    nc.scalar.activation(h1[:, :], z1s[th][:, :], TANH,
                                         bias=wtt[:, n:n + 1])
                    h1s.append(h1)
                z2s = [zpool.tile([128, W], F32, tag="z", name=f"z2_{th}")
                       for th in range(NTH)]
                for th in range(NTH):
                    for sl in csl:
                        nc.tensor.matmul(z2s[th][:, sl], w2t[:, :],
                                         h1s[th][:, sl],
                                         start=True, stop=True)
                h2s = []
                for th in range(NTH):
                    h2 = h2pool.tile([128, W], F32R, tag="h2", name=f"h2_{th}")
                    nc.scalar.activation(h2[:, :], z2s[th][:, :], TANH,
                                         bias=b2t[:, 0:1])
                    h2s.append(h2)
                for th in range(NTH):
                    for sl in csl:
                        nc.tensor.matmul(ypsum[th][:, sl], w3_s[s][:, :],
                                         h2s[th][:, sl],
                                         start=(s == 0), stop=(s == 3))
                h2prev = h2s

            # y_{n+1} = ypsum + y_n (+ h*b3): exact fp32 on DVE
            for th in range(NTH):
                nc.vector.tensor_add(xb[th][nxt][0:3, :], ypsum[th][:, :],
                                     xb[th][par][0:3, :])
                if has_b3:
                    nc.vector.tensor_scalar_add(xb[th][nxt][0:3, :],
                                                xb[th][nxt][0:3, :],
                                                hb3t[:, 0:1])
                nc.sync.dma_start(yout[:, n, th * W:(th + 1) * W],
                                  xb[th][nxt][0:3, :])


def build_program(B_core, T, NTH, has_b3=False, debug=False,
                  enable_asserts=False):
    nc = bacc.Bacc("TRN2", target_bir_lowering=False, debug=debug,
                   enable_asserts=enable_asserts, num_devices=1)
    shapes = {
        "xinit": ((17, B_core), F32R),
        "uT": ((T * 4, B_core), F32R),
        "Wc1": ((17, 128), F32R), "Wc23": ((17, 128), F32R),
        "Wc4": ((17, 128), F32R),
        "Ch": ((128, 128), F32R), "Cf": ((128, 128), F32R),
        "W2T": ((128, 128), F32R),
        "W36": ((128, 3), F32R), "W33": ((128, 3), F32R),
        "wtt": ((128, T), F32), "b2": ((128, 1), F32),
    }
    if has_b3:
        shapes["hb3"] = ((3, 1), F32)
    aps = {}
    for name, (shp, dt) in shapes.items():
        aps[name] = nc.dram_tensor(name, list(shp), dt,
                                   kind="ExternalInput").ap()
    aps["yout"] = nc.dram_tensor("yout", [3, T - 1, B_core], F32R,
                                 kind="ExternalOutput").ap()
    with tile.TileContext(nc) as tc:
        build_tile_body(tc, aps, B_core, T, NTH, has_b3)
    nc.compile()
    return nc


def make_in_maps(y0, t, u, p, W1, b1, W2, b2, W3, b3, n_cores, B_core, T,
                 has_b3):
    f32 = np.float32
    y0 = np.asarray(y0, f32); u = np.asarray(u, f32); p = np.asarray(p, f32)
    consts = prepare_consts(W1, b1, W2, b2, W3, b3, t)
    if not has_b3:
        consts.pop("hb3")
    in_maps = []
    for i in range(n_cores):
        sl = slice(i * B_core, (i + 1) * B_core)
        xinit = np.zeros((17, B_core), f32)
        xinit[0:3] = y0[sl].T
        xinit[3] = 1.0
        xinit[4:9] = p[sl].T
        uT = np.ascontiguousarray(
            u[sl].transpose(1, 2, 0).reshape(T * 4, B_core))
        m = {"xinit": xinit, "uT": uT}
        m.update(consts)
        in_maps.append(m)
    return in_maps


_PROGRAM_CACHE = {}


def _get_program(B_core, T, NTH, has_b3):
    key = (B_core, T, NTH, has_b3)
    if key not in _PROGRAM_CACHE:
        _PROGRAM_CACHE[key] = build_program(B_core, T, NTH, has_b3)
    return _PROGRAM_CACHE[key]


def run_on_cores(inputs, n_cores=N_CORES, NTH=2, trace=False):
    y0 = np.asarray(inputs["y0"], np.float32)
    B = y0.shape[0]
    T = np.asarray(inputs["t"]).shape[0]
    B_core = B // n_cores
    has_b3 = bool(np.any(np.asarray(inputs["b3"]) != 0))
    nc = _get_program(B_core, T, NTH, has_b3)
    in_maps = make_in_maps(
        inputs["y0"], inputs["t"], inputs["u"], inputs["p"],
        inputs["W1"], inputs["b1"], inputs["W2"], inputs["b2"],
        inputs["W3"], inputs["b3"], n_cores, B_core, T, has_b3)
    res = run_bass_kernel_spmd(nc, in_maps, list(range(n_cores)), trace=trace)
    out = np.empty((B, T, 3), np.float32)
    for i in range(n_cores):
        sl = slice(i * B_core, (i + 1) * B_core)
        yo = np.asarray(res.results[i]["yout"])        # (3, T-1, B_core)
        out[sl, 1:, :] = yo.transpose(2, 1, 0)
        out[sl, 0, :] = y0[sl]
    return out, res


def kernel(y0, t, u, p, W1, b1, W2, b2, W3, b3):
    out, _ = run_on_cores(
        dict(y0=y0, t=t, u=u, p=p, W1=W1, b1=b1, W2=W2, b2=b2,
             W3=W3, b3=b3),
        n_cores=N_CORES, NTH=2, trace=False)
    return out



# revision 12
# speedup vs baseline: 1.6235x; 1.6235x over previous
"""Trainium2 Bass kernel for NeuralBlochRK4.

Reference computation: RK4 integration (255 steps) of dy/dt = MLP([y,u(t),p,t])
with MLP 13 -> 128(tanh) -> 128(tanh) -> 3, batch 16384, output = full
trajectory (B, 256, 3).

Strategy (pure data-parallel over batch, 8 cores x 2048 rows):
  * fp16 matmul operands (single-pass PE, fast weight load); fp32 PSUM
    accumulation; the integrated state y kept fp32 end to end.
  * Packed input tile x = [y_old(3); ones(1); p(5); u_n(4); u_end(4);
    y_new(3)] (K=20).  Stages 2/3 read y_new = y_n; stages 0/1 of the NEXT
    step read y_old = y_n and receive the A@(y_{n+1}-y_n) increment through
    their C-family matmul (C6 @ hs for stage 0, Ch @ (hs/3 + h2_0) for
    stage 1, hs = h2_0+2h2_1+2h2_2+h2_3).  This keeps every tanh fed without
    waiting on the DVE y-update chain at the step boundary.
  * Per stage, z1 = Wc_s^T @ x + C-term accumulated in PSUM; tanh on ACT with
    per-step bias w_t*t_n.  z2 = W2 @ h1; tanh, bias b2.  y_{n+1} increment =
    W36 @ hs (one pass); one DVE add folds in y_n (fp32 state tile).
  * Next-stage K20 matmuls are hoisted into the previous stage's ACT window;
    the z/y PSUM tiles share one 4-slot ring (8 banks) whose reuse hazards
    all coincide with true data dependencies.
  * Batch split into 2 interleaved "threads" of 1024 per core so ACT/PE
    pipeline across threads.
"""

import numpy as np
from contextlib import ExitStack

import concourse.bass as bass
import concourse.tile as tile
from concourse import bacc, mybir
from concourse.bass_utils import run_bass_kernel_spmd

F32 = mybir.dt.float32
F16 = mybir.dt.float16
TANH = mybir.ActivationFunctionType.Tanh

B_FULL, T_FULL, HID = 16384, 256, 128
N_CORES = 8

KX = 35  # packed x rows: y_old 0:3, ones 3, p 4:9, u 9:17, y_new 32:35
YN = 32  # y_new base partition (DVE writes need 32-aligned bases)


# ----------------------------------------------------------------------------
# host-side constant preparation
# ----------------------------------------------------------------------------

def prepare_consts(W1, b1, W2, b2, W3, b3, t):
    f32 = np.float32
    f16 = np.float16
    W1 = np.asarray(W1, f32); W2 = np.asarray(W2, f32); W3 = np.asarray(W3, f32)
    b1 = np.asarray(b1, f32); b2 = np.asarray(b2, f32); b3 = np.asarray(b3, f32)
    t = np.asarray(t, f32)
    h = f32(t[1] - t[0])

    A = W1[:, 0:3]
    U = W1[:, 3:7]
    P = W1[:, 7:12]
    w_t = W1[:, 12]
    C = (A @ W3).astype(f32)
    Ab3 = (A @ b3).astype(f32)

    def wc(y_rows, bias, cn, ce):
        kxm = np.zeros((KX, 128), f32)
        kxm[y_rows:y_rows + 3, :] = A.T
        kxm[3, :] = bias
        kxm[4:9, :] = P.T
        kxm[9:13, :] = cn * U.T
        kxm[13:17, :] = ce * U.T
        return np.ascontiguousarray(kxm.astype(f16))

    consts = {
        # prologue stage 0 (true y_0 in y_new rows)
        "Wc1p": wc(YN, b1, 1.0, 0.0),
        # chained stage 0: y_old rows; A@(y_{n+1}-y_n) via C6@hs
        "Wc1c": wc(0, b1 + h * Ab3, 1.0, 0.0),
        # stage 1 of step 0 (normal, y_new rows)
        "Wc23n": wc(YN, b1 + w_t * (h / 2) + (h / 2) * Ab3, 0.5, 0.5),
        # chained stage 1: y_old rows; increment via Ch@(hs/3 + h2_0)
        "Wc23o": wc(0, b1 + w_t * (h / 2) + (3 * h / 2) * Ab3, 0.5, 0.5),
        # stage 2 (y_new rows, normal)
        "Wc23b": wc(YN, b1 + w_t * (h / 2) + (h / 2) * Ab3, 0.5, 0.5),
        # stage 3 (y_new rows, normal)
        "Wc4": wc(YN, b1 + w_t * h + h * Ab3, 0.0, 1.0),
        "Ch": np.ascontiguousarray((f32(h / 2) * C.T).astype(f16)),
        "Cf": np.ascontiguousarray((f32(h) * C.T).astype(f16)),
        "C6": np.ascontiguousarray((f32(h / 6) * C.T).astype(f16)),
        "W2T": np.ascontiguousarray(W2.T.astype(f16)),
        "W36": np.ascontiguousarray((f32(h / 6) * W3.T).astype(f16)),
        "wtt": np.ascontiguousarray(np.outer(w_t, t).astype(f32)),
        "b2": np.ascontiguousarray(b2.reshape(128, 1)),
        "hb3": np.ascontiguousarray((h * b3).reshape(3, 1)),
    }
    return consts


# ----------------------------------------------------------------------------
# device program
# ----------------------------------------------------------------------------

def build_tile_body(tc, aps, B_core, T, NTH, has_b3):
    nc = tc.nc
    W = B_core // NTH          # per-thread batch width
    CH = min(512, W)           # matmul free-dim chunk (one PSUM bank)
    NCH = W // CH
    assert W % CH == 0 and B_core % NTH == 0
    assert NTH == 2 and NCH == 2

    with ExitStack() as ctx:
        wpool = ctx.enter_context(tc.tile_pool(name="wts", bufs=1))
        xpool = ctx.enter_context(tc.tile_pool(name="x", bufs=1))
        yspool = ctx.enter_context(tc.tile_pool(name="ys", bufs=1))
        h1pool = ctx.enter_context(tc.tile_pool(name="h1", bufs=2))
        h2pool = ctx.enter_context(tc.tile_pool(name="h2", bufs=8))
        qpool = ctx.enter_context(tc.tile_pool(name="q", bufs=2))
        zpool = ctx.enter_context(
            tc.tile_pool(name="z", bufs=4, space=bass.MemorySpace.PSUM))

        def wtile(name, shape, dt):
            tl = wpool.tile(list(shape), dt, tag=name)
            nc.sync.dma_start(tl[:, :], aps[name][:, :])
            return tl

        wc1p = wtile("Wc1p", (KX, 128), F16)
        wc1c = wtile("Wc1c", (KX, 128), F16)
        wc23n = wtile("Wc23n", (KX, 128), F16)
        wc23o = wtile("Wc23o", (KX, 128), F16)
        wc23b = wtile("Wc23b", (KX, 128), F16)
        wc4 = wtile("Wc4", (KX, 128), F16)
        ch_t = wtile("Ch", (128, 128), F16)
        cf_t = wtile("Cf", (128, 128), F16)
        c6_t = wtile("C6", (128, 128), F16)
        w2t = wtile("W2T", (128, 128), F16)
        w36 = wtile("W36", (128, 3), F16)
        wtt = wtile("wtt", (128, T), F32)
        b2t = wtile("b2", (128, 1), F32)
        hb3t = wtile("hb3", (3, 1), F32) if has_b3 else None

        yout = aps["yout"]      # (3, T-1, B_core) f32
        uT = aps["uT"]          # (T*4, B_core)   f16
        xinit = aps["xinit"]    # (KX, B_core)    f16
        y0T = aps["y0T"]        # (3, B_core)     f32

        # persistent x tiles [thread][parity]; fp32 y state [thread][parity]
        xb, ys = [], []
        for th in range(NTH):
            bufs = []
            for par in range(2):
                tl = xpool.tile([KX, W], F16, tag=f"xb{th}{par}")
                nc.sync.dma_start(tl[:, :], xinit[:, th * W:(th + 1) * W])
                bufs.append(tl)
            xb.append(bufs)
            ybufs = []
            for par in range(2):
                tl = yspool.tile([3, W], F32, tag=f"ys{th}{par}")
                if par == 0:
                    nc.sync.dma_start(tl[:, :], y0T[:, th * W:(th + 1) * W])
                ybufs.append(tl)
            ys.append(ybufs)
        for th in range(NTH):
            nc.sync.dma_start(xb[th][0][9:17, :], uT[0:8, th * W:(th + 1) * W])
            if T - 1 > 1:
                nc.sync.dma_start(xb[th][1][9:17, :], uT[4:12, th * W:(th + 1) * W])

        csl = [slice(c * CH, (c + 1) * CH) for c in range(NCH)]

        def zalloc(name):
            return zpool.tile([128, W], F32, tag="z", name=name)

        def yalloc(name):
            return zpool.tile([3, W], F32, tag="z", name=name,
                              padded_shape=[128, W])

        # prologue: stage-0 K20 matmuls for step 0 (true y_0)
        z1s = [zalloc(f"z1_{th}") for th in range(NTH)]
        for th in range(NTH):
            for sl in csl:
                nc.tensor.matmul(z1s[th][:, sl], wc1p[:, :], xb[th][0][:, sl],
                                 start=True, stop=True)

        hs3_prev = [None] * NTH  # hs/3 tiles from the previous step
        tc_pending = None        # deferred boundary x y_new writes

        for n in range(T - 1):
            par, nxt = n % 2, (n + 1) % 2
            last = (n == T - 2)

            if not last:
                r0 = 4 * (n + 1)
                for th in range(NTH):
                    nc.sync.dma_start(xb[th][nxt][9:17, :],
                                      uT[r0:r0 + 8, th * W:(th + 1) * W])

            h2keep = [[None] * 4 for _ in range(NTH)]
            qs = [None] * NTH

            for s in range(4):
                # tanh layer 1 (z1 psum groups were completed earlier)
                h1s = []
                for th in range(NTH):
                    h1 = h1pool.tile([128, W], F16, tag="h1", name=f"h1_{th}")
                    nc.scalar.activation(h1[:, :], z1s[th][:, :], TANH,
                                         bias=wtt[:, n:n + 1])
                    h1s.append(h1)
                # z2 matmuls
                z2s = [zalloc(f"z2_{th}") for th in range(NTH)]
                for th in range(NTH):
                    for sl in csl:
                        nc.tensor.matmul(z2s[th][:, sl], w2t[:, :],
                                         h1s[th][:, sl],
                                         start=True, stop=True)
                if s == 3 and not last:
                    # hoist next step's stage-0 K20 on [y_n(old); p; u_{n+1}]
                    z1n = [zalloc(f"z1_{th}") for th in range(NTH)]
                    for th in range(NTH):
                        for sl in csl:
                            nc.tensor.matmul(z1n[th][:, sl], wc1c[:, :],
                                             xb[th][nxt][:, sl],
                                             start=True, stop=False)
                # tanh layer 2
                h2s = []
                for th in range(NTH):
                    h2 = h2pool.tile([128, W], F16, tag="h2", name=f"h2_{th}")
                    nc.scalar.activation(h2[:, :], z2s[th][:, :], TANH,
                                         bias=b2t[:, 0:1])
                    h2s.append(h2)
                    h2keep[th][s] = h2

                if s == 0:
                    # hoist stage 1: K20 (+ C-term via m1 = hs/3 + h2_0 for
                    # chained steps, direct Ch @ h2_0 for step 0)
                    z1n = [zalloc(f"z1_{th}") for th in range(NTH)]
                    w_s1 = wc23n if n == 0 else wc23o
                    for th in range(NTH):
                        for sl in csl:
                            nc.tensor.matmul(z1n[th][:, sl], w_s1[:, :],
                                             xb[th][par][:, sl],
                                             start=True, stop=False)
                    for th in range(NTH):
                        if n == 0:
                            cmov = h2s[th]
                        else:
                            m1 = qpool.tile([128, W], F16, tag=f"m1{th}",
                                            name="m1")
                            nc.vector.tensor_add(m1[:, :],
                                                 hs3_prev[th][:, :],
                                                 h2s[th][:, :])
                            cmov = m1
                        for sl in csl:
                            nc.tensor.matmul(z1n[th][:, sl], ch_t[:, :],
                                             cmov[:, sl],
                                             start=False, stop=True)
                    z1s = z1n
                    # deferred boundary write of y_new rows (kept off the
                    # stage-1 critical path)
                    if tc_pending is not None:
                        for th in range(NTH):
                            nc.vector.tensor_copy(tc_pending[th][0][YN:YN + 3, :],
                                                  tc_pending[th][1][:, :])
                        tc_pending = None
                elif s == 1:
                    # copy y_n into the next x tile's y_old rows (cheap fp16
                    # move, far off the critical path)
                    if not last:
                        for th in range(NTH):
                            nc.vector.tensor_copy(xb[th][nxt][0:3, :],
                                                  xb[th][par][YN:YN + 3, :])
                    z1n = [zalloc(f"z1_{th}") for th in range(NTH)]
                    for th in range(NTH):
                        for sl in csl:
                            nc.tensor.matmul(z1n[th][:, sl], wc23b[:, :],
                                             xb[th][par][:, sl],
                                             start=True, stop=False)
                    for th in range(NTH):
                        for sl in csl:
                            nc.tensor.matmul(z1n[th][:, sl], ch_t[:, :],
                                             h2s[th][:, sl],
                                             start=False, stop=True)
                    z1s = z1n
                elif s == 2:
                    # q = h2_0 + 2*(h2_1 + h2_2) on DVE (off-critical-path)
                    for th in range(NTH):
                        e = qpool.tile([128, W], F16, tag=f"e{th}", name="e")
                        nc.vector.tensor_add(e[:, :], h2keep[th][1][:, :],
                                             h2keep[th][2][:, :])
                        g = qpool.tile([128, W], F16, tag=f"g{th}", name="g")
                        nc.vector.tensor_scalar_mul(g[:, :], e[:, :], 2.0)
                        q = qpool.tile([128, W], F16, tag=f"q{th}", name="q")
                        nc.vector.tensor_add(q[:, :], g[:, :],
                                             h2keep[th][0][:, :])
                        qs[th] = q
                    z1n = [zalloc(f"z1_{th}") for th in range(NTH)]
                    for th in range(NTH):
                        for sl in csl:
                            nc.tensor.matmul(z1n[th][:, sl], wc4[:, :],
                                             xb[th][par][:, sl],
                                             start=True, stop=False)
                    for th in range(NTH):
                        for sl in csl:
                            nc.tensor.matmul(z1n[th][:, sl], cf_t[:, :],
                                             h2s[th][:, sl],
                                             start=False, stop=True)
                    z1s = z1n
                else:
                    # boundary: hs = q + h2_3; finish hoisted stage-0 group
                    # with C6 @ hs; y increment via W36 @ hs; fp32 state
                    # update on DVE; hs/3 for the next step's stage 1.
                    hss = []
                    for th in range(NTH):
                        hs = qpool.tile([128, W], F16, tag=f"hs{th}",
                                        name="hs")
                        nc.vector.tensor_add(hs[:, :], qs[th][:, :],
                                             h2s[th][:, :])
                        hss.append(hs)
                    if not last:
                        for th in range(NTH):
                            for sl in csl:
                                nc.tensor.matmul(z1n[th][:, sl], c6_t[:, :],
                                                 hss[th][:, sl],
                                                 start=False, stop=True)
                        z1s = z1n
                        for th in range(NTH):
                            hs3 = qpool.tile([128, W], F16, tag=f"hs3{th}",
                                             name="hs3")
                            nc.vector.tensor_scalar_mul(hs3[:, :],
                                                        hss[th][:, :],
                                                        1.0 / 3.0)
                            hs3_prev[th] = hs3
                    ypsum = [yalloc(f"yp{th}") for th in range(NTH)]
                    for th in range(NTH):
                        for sl in csl:
                            nc.tensor.matmul(ypsum[th][0:3, sl], w36[:, :],
                                             hss[th][:, sl],
                                             start=True, stop=True)
                    for th in range(NTH):
                        nc.vector.tensor_add(ys[th][nxt][:, :],
                                             ypsum[th][0:3, :],
                                             ys[th][par][:, :])
                        if has_b3:
                            nc.vector.tensor_scalar_add(ys[th][nxt][:, :],
                                                        ys[th][nxt][:, :],
                                                        hb3t[:, 0:1])
                        nc.sync.dma_start(yout[:, n, th * W:(th + 1) * W],
                                          ys[th][nxt][:, :])
                    # y_new rows of the next x tile are written early in the
                    # next step (deferred so stage-1's m1 DVE op goes first)
                    if not last:
                        tc_pending = [(xb[th][nxt], ys[th][nxt])
                                      for th in range(NTH)]


def build_program(B_core, T, NTH, has_b3=False, debug=False,
                  enable_asserts=False):
    nc = bacc.Bacc("TRN2", target_bir_lowering=False, debug=debug,
                   enable_asserts=enable_asserts, num_devices=1)
    shapes = {
        "xinit": ((KX, B_core), F16),
        "uT": ((T * 4, B_core), F16),
        "y0T": ((3, B_core), F32),
        "Wc1p": ((KX, 128), F16), "Wc1c": ((KX, 128), F16),
        "Wc23n": ((KX, 128), F16), "Wc23o": ((KX, 128), F16),
        "Wc23b": ((KX, 128), F16), "Wc4": ((KX, 128), F16),
        "Ch": ((128, 128), F16), "Cf": ((128, 128), F16),
        "C6": ((128, 128), F16), "W2T": ((128, 128), F16),
        "W36": ((128, 3), F16),
        "wtt": ((128, T), F32), "b2": ((128, 1), F32),
    }
    if has_b3:
        shapes["hb3"] = ((3, 1), F32)
    aps = {}
    for name, (shp, dt) in shapes.items():
        aps[name] = nc.dram_tensor(name, list(shp), dt,
                                   kind="ExternalInput").ap()
    aps["yout"] = nc.dram_tensor("yout", [3, T - 1, B_core], F32,
                                 kind="ExternalOutput").ap()
    with tile.TileContext(nc) as tc:
        build_tile_body(tc, aps, B_core, T, NTH, has_b3)
    nc.compile()
    return nc


def make_in_maps(y0, t, u, p, W1, b1, W2, b2, W3, b3, n_cores, B_core, T,
                 has_b3):
    f32 = np.float32
    f16 = np.float16
    y0 = np.asarray(y0, f32); u = np.asarray(u, f32); p = np.asarray(p, f32)
    consts = prepare_consts(W1, b1, W2, b2, W3, b3, t)
    if not has_b3:
        consts.pop("hb3")
    in_maps = []
    for i in range(n_cores):
        sl = slice(i * B_core, (i + 1) * B_core)
        xinit = np.zeros((KX, B_core), f16)
        xinit[0:3] = y0[sl].T.astype(f16)
        xinit[3] = 1.0
        xinit[4:9] = p[sl].T.astype(f16)
        xinit[YN:YN + 3] = y0[sl].T.astype(f16)
        uT = np.ascontiguousarray(
            u[sl].transpose(1, 2, 0).reshape(T * 4, B_core).astype(f16))
        y0T = np.ascontiguousarray(y0[sl].T)
        m = {"xinit": xinit, "uT": uT, "y0T": y0T}
        m.update(consts)
        in_maps.append(m)
    return in_maps


_PROGRAM_CACHE = {}


def _get_program(B_core, T, NTH, has_b3):
    key = (B_core, T, NTH, has_b3)
    if key not in _PROGRAM_CACHE:
        _PROGRAM_CACHE[key] = build_program(B_core, T, NTH, has_b3)
    return _PROGRAM_CACHE[key]


def run_on_cores(inputs, n_cores=N_CORES, NTH=2, trace=False):
    y0 = np.asarray(inputs["y0"], np.float32)
    B = y0.shape[0]
    T = np.asarray(inputs["t"]).shape[0]
    B_core = B // n_cores
    has_b3 = bool(np.any(np.asarray(inputs["b3"]) != 0))
    nc = _get_program(B_core, T, NTH, has_b3)
    in_maps = make_in_maps(
        inputs["y0"], inputs["t"], inputs["u"], inputs["p"],
        inputs["W1"], inputs["b1"], inputs["W2"], inputs["b2"],
        inputs["W3"], inputs["b3"], n_cores, B_core, T, has_b3)
    res = run_bass_kernel_spmd(nc, in_maps, list(range(n_cores)), trace=trace)
    out = np.empty((B, T, 3), np.float32)
    for i in range(n_cores):
        sl = slice(i * B_core, (i + 1) * B_core)
        yo = np.asarray(res.results[i]["yout"])        # (3, T-1, B_core)
        out[sl, 1:, :] = yo.transpose(2, 1, 0)
        out[sl, 0, :] = y0[sl]
    return out, res


def kernel(y0, t, u, p, W1, b1, W2, b2, W3, b3):
    out, _ = run_on_cores(
        dict(y0=y0, t=t, u=u, p=p, W1=W1, b1=b1, W2=W2, b2=b2,
             W3=W3, b3=b3),
        n_cores=N_CORES, NTH=2, trace=False)
    return out


# revision 13
# speedup vs baseline: 1.6861x; 1.0386x over previous
"""Trainium2 Bass kernel for NeuralBlochRK4.

Reference computation: RK4 integration (255 steps) of dy/dt = MLP([y,u(t),p,t])
with MLP 13 -> 128(tanh) -> 128(tanh) -> 3, batch 16384, output = full
trajectory (B, 256, 3).

Strategy (pure data-parallel over batch, 8 cores x 2048 rows):
  * fp16 matmul operands (single-pass PE, fast weight load); fp32 PSUM
    accumulation; the integrated state y kept fp32 end to end.
  * Packed input tile x = [y_old(3); ones(1); p(5); u_n(4); u_end(4);
    y_new(3)] (K=20).  Stages 2/3 read y_new = y_n; stages 0/1 of the NEXT
    step read y_old = y_n and receive the A@(y_{n+1}-y_n) increment through
    their C-family matmul (C6 @ hs for stage 0, Ch @ (hs/3 + h2_0) for
    stage 1, hs = h2_0+2h2_1+2h2_2+h2_3).  This keeps every tanh fed without
    waiting on the DVE y-update chain at the step boundary.
  * Per stage, z1 = Wc_s^T @ x + C-term accumulated in PSUM; tanh on ACT with
    per-step bias w_t*t_n.  z2 = W2 @ h1; tanh, bias b2.  y_{n+1} increment =
    W36 @ hs (one pass); one DVE add folds in y_n (fp32 state tile).
  * Next-stage K20 matmuls are hoisted into the previous stage's ACT window;
    the z/y PSUM tiles share one 4-slot ring (8 banks) whose reuse hazards
    all coincide with true data dependencies.
  * Batch split into 2 interleaved "threads" of 1024 per core so ACT/PE
    pipeline across threads.
"""

import numpy as np
from contextlib import ExitStack

import concourse.bass as bass
import concourse.tile as tile
from concourse import bacc, mybir
from concourse.bass_utils import run_bass_kernel_spmd

F32 = mybir.dt.float32
F16 = mybir.dt.float16
TANH = mybir.ActivationFunctionType.Tanh

B_FULL, T_FULL, HID = 16384, 256, 128
N_CORES = 8

KX = 35  # packed x rows: y_old 0:3, ones 3, p 4:9, u 9:17, y_new 32:35
YN = 32  # y_new base partition (DVE writes need 32-aligned bases)


# ----------------------------------------------------------------------------
# host-side constant preparation
# ----------------------------------------------------------------------------

def prepare_consts(W1, b1, W2, b2, W3, b3, t):
    f32 = np.float32
    f16 = np.float16
    W1 = np.asarray(W1, f32); W2 = np.asarray(W2, f32); W3 = np.asarray(W3, f32)
    b1 = np.asarray(b1, f32); b2 = np.asarray(b2, f32); b3 = np.asarray(b3, f32)
    t = np.asarray(t, f32)
    h = f32(t[1] - t[0])

    A = W1[:, 0:3]
    U = W1[:, 3:7]
    P = W1[:, 7:12]
    w_t = W1[:, 12]
    C = (A @ W3).astype(f32)
    Ab3 = (A @ b3).astype(f32)

    def wc(y_rows, bias, cn, ce):
        kxm = np.zeros((KX, 128), f32)
        kxm[y_rows:y_rows + 3, :] = A.T
        kxm[3, :] = bias
        kxm[4:9, :] = P.T
        kxm[9:13, :] = cn * U.T
        kxm[13:17, :] = ce * U.T
        return np.ascontiguousarray(kxm.astype(f16))

    consts = {
        # prologue stage 0 (true y_0 in y_new rows)
        "Wc1p": wc(YN, b1, 1.0, 0.0),
        # chained stage 0: y_old rows; A@(y_{n+1}-y_n) via C6@hs
        "Wc1c": wc(0, b1 + h * Ab3, 1.0, 0.0),
        # stage 1 of step 0 (normal, y_new rows)
        "Wc23n": wc(YN, b1 + w_t * (h / 2) + (h / 2) * Ab3, 0.5, 0.5),
        # chained stage 1: y_old rows; increment via Ch@(hs/3 + h2_0)
        "Wc23o": wc(0, b1 + w_t * (h / 2) + (3 * h / 2) * Ab3, 0.5, 0.5),
        # stage 2 (y_new rows, normal)
        "Wc23b": wc(YN, b1 + w_t * (h / 2) + (h / 2) * Ab3, 0.5, 0.5),
        # stage 3 (y_new rows, normal)
        "Wc4": wc(YN, b1 + w_t * h + h * Ab3, 0.0, 1.0),
        "Ch": np.ascontiguousarray((f32(h / 2) * C.T).astype(f16)),
        "Cf": np.ascontiguousarray((f32(h) * C.T).astype(f16)),
        "C6": np.ascontiguousarray((f32(h / 6) * C.T).astype(f16)),
        "W2T": np.ascontiguousarray(W2.T.astype(f16)),
        "W36": np.ascontiguousarray((f32(h / 6) * W3.T).astype(f16)),
        "wtt": np.ascontiguousarray(np.outer(w_t, t).astype(f32)),
        "b2": np.ascontiguousarray(b2.reshape(128, 1)),
        "hb3": np.ascontiguousarray((h * b3).reshape(3, 1)),
    }
    return consts


# ----------------------------------------------------------------------------
# device program
# ----------------------------------------------------------------------------

def build_tile_body(tc, aps, B_core, T, NTH, has_b3):
    nc = tc.nc
    W = B_core // NTH          # per-thread batch width
    CH = min(512, W)           # matmul free-dim chunk (one PSUM bank)
    NCH = W // CH
    assert W % CH == 0 and B_core % NTH == 0
    assert NTH == 2 and NCH == 2

    with ExitStack() as ctx:
        wpool = ctx.enter_context(tc.tile_pool(name="wts", bufs=1))
        xpool = ctx.enter_context(tc.tile_pool(name="x", bufs=1))
        yspool = ctx.enter_context(tc.tile_pool(name="ys", bufs=1))
        h1pool = ctx.enter_context(tc.tile_pool(name="h1", bufs=2))
        h2pool = ctx.enter_context(tc.tile_pool(name="h2", bufs=8))
        qpool = ctx.enter_context(tc.tile_pool(name="q", bufs=2))
        zpool = ctx.enter_context(
            tc.tile_pool(name="z", bufs=4, space=bass.MemorySpace.PSUM))

        def wtile(name, shape, dt):
            tl = wpool.tile(list(shape), dt, tag=name)
            nc.sync.dma_start(tl[:, :], aps[name][:, :])
            return tl

        wc1p = wtile("Wc1p", (KX, 128), F16)
        wc1c = wtile("Wc1c", (KX, 128), F16)
        wc23n = wtile("Wc23n", (KX, 128), F16)
        wc23o = wtile("Wc23o", (KX, 128), F16)
        wc23b = wtile("Wc23b", (KX, 128), F16)
        wc4 = wtile("Wc4", (KX, 128), F16)
        ch_t = wtile("Ch", (128, 128), F16)
        cf_t = wtile("Cf", (128, 128), F16)
        c6_t = wtile("C6", (128, 128), F16)
        w2t = wtile("W2T", (128, 128), F16)
        w36 = wtile("W36", (128, 3), F16)
        wtt = wtile("wtt", (128, T), F32)
        b2t = wtile("b2", (128, 1), F32)
        hb3t = wtile("hb3", (3, 1), F32) if has_b3 else None

        yout = aps["yout"]      # (3, T-1, B_core) f32
        uT = aps["uT"]          # (T*4, B_core)   f16
        xinit = aps["xinit"]    # (KX, B_core)    f16
        y0T = aps["y0T"]        # (3, B_core)     f32

        # persistent x tiles [thread][parity]; fp32 y state [thread][parity]
        xb, ys = [], []
        for th in range(NTH):
            bufs = []
            for par in range(2):
                tl = xpool.tile([KX, W], F16, tag=f"xb{th}{par}")
                nc.sync.dma_start(tl[:, :], xinit[:, th * W:(th + 1) * W])
                bufs.append(tl)
            xb.append(bufs)
            ybufs = []
            for par in range(2):
                tl = yspool.tile([3, W], F32, tag=f"ys{th}{par}")
                if par == 0:
                    nc.sync.dma_start(tl[:, :], y0T[:, th * W:(th + 1) * W])
                ybufs.append(tl)
            ys.append(ybufs)
        for th in range(NTH):
            nc.sync.dma_start(xb[th][0][9:17, :], uT[0:8, th * W:(th + 1) * W])
            if T - 1 > 1:
                nc.sync.dma_start(xb[th][1][9:17, :], uT[4:12, th * W:(th + 1) * W])

        csl = [slice(c * CH, (c + 1) * CH) for c in range(NCH)]

        def zalloc(name):
            return zpool.tile([128, W], F32, tag="z", name=name)

        def yalloc(name):
            return zpool.tile([3, W], F32, tag="z", name=name,
                              padded_shape=[128, W])

        # prologue: stage-0 K20 matmuls for step 0 (true y_0)
        z1s = [zalloc(f"z1_{th}") for th in range(NTH)]
        for th in range(NTH):
            for sl in csl:
                nc.tensor.matmul(z1s[th][:, sl], wc1p[:, :], xb[th][0][:, sl],
                                 start=True, stop=True)

        hs_prev = [None] * NTH  # hs tiles from the previous step
        tc_pending = None        # deferred boundary x y_new writes

        for n in range(T - 1):
            par, nxt = n % 2, (n + 1) % 2
            last = (n == T - 2)

            if not last:
                r0 = 4 * (n + 1)
                for th in range(NTH):
                    nc.sync.dma_start(xb[th][nxt][9:17, :],
                                      uT[r0:r0 + 8, th * W:(th + 1) * W])

            h2keep = [[None] * 4 for _ in range(NTH)]
            qs = [None] * NTH

            for s in range(4):
                # tanh layer 1 (z1 psum groups were completed earlier)
                h1s = []
                for th in range(NTH):
                    h1 = h1pool.tile([128, W], F16, tag="h1", name=f"h1_{th}")
                    nc.scalar.activation(h1[:, :], z1s[th][:, :], TANH,
                                         bias=wtt[:, n:n + 1])
                    h1s.append(h1)
                # z2 matmuls
                z2s = [zalloc(f"z2_{th}") for th in range(NTH)]
                for th in range(NTH):
                    for sl in csl:
                        nc.tensor.matmul(z2s[th][:, sl], w2t[:, :],
                                         h1s[th][:, sl],
                                         start=True, stop=True)
                if s == 3 and not last:
                    # hoist next step's stage-0: K35 on [y_n(old); p; u_{n+1}]
                    # plus C6 @ q (ready since stage 2)
                    z1n = [zalloc(f"z1_{th}") for th in range(NTH)]
                    for th in range(NTH):
                        for sl in csl:
                            nc.tensor.matmul(z1n[th][:, sl], wc1c[:, :],
                                             xb[th][nxt][:, sl],
                                             start=True, stop=False)
                    for th in range(NTH):
                        for sl in csl:
                            nc.tensor.matmul(z1n[th][:, sl], c6_t[:, :],
                                             qs[th][:, sl],
                                             start=False, stop=False)
                # tanh layer 2
                h2s = []
                for th in range(NTH):
                    h2 = h2pool.tile([128, W], F16, tag="h2", name=f"h2_{th}")
                    nc.scalar.activation(h2[:, :], z2s[th][:, :], TANH,
                                         bias=b2t[:, 0:1])
                    h2s.append(h2)
                    h2keep[th][s] = h2

                if s == 0:
                    # hoist stage 1: K35 + C6@hs_prev (both ready now) with
                    # the only chained term Ch @ h2_0 (no DVE on this path)
                    z1n = [zalloc(f"z1_{th}") for th in range(NTH)]
                    w_s1 = wc23n if n == 0 else wc23o
                    for th in range(NTH):
                        for sl in csl:
                            nc.tensor.matmul(z1n[th][:, sl], w_s1[:, :],
                                             xb[th][par][:, sl],
                                             start=True, stop=False)
                    if n > 0:
                        for th in range(NTH):
                            for sl in csl:
                                nc.tensor.matmul(z1n[th][:, sl], c6_t[:, :],
                                                 hs_prev[th][:, sl],
                                                 start=False, stop=False)
                    for th in range(NTH):
                        for sl in csl:
                            nc.tensor.matmul(z1n[th][:, sl], ch_t[:, :],
                                             h2s[th][:, sl],
                                             start=False, stop=True)
                    z1s = z1n
                    # deferred boundary write of y_new rows (kept off the
                    # stage-1 critical path)
                    if tc_pending is not None:
                        for th in range(NTH):
                            nc.vector.tensor_copy(tc_pending[th][0][YN:YN + 3, :],
                                                  tc_pending[th][1][:, :])
                        tc_pending = None
                elif s == 1:
                    # copy y_n into the next x tile's y_old rows (cheap fp16
                    # move, far off the critical path)
                    if not last:
                        for th in range(NTH):
                            nc.vector.tensor_copy(xb[th][nxt][0:3, :],
                                                  xb[th][par][YN:YN + 3, :])
                    z1n = [zalloc(f"z1_{th}") for th in range(NTH)]
                    for th in range(NTH):
                        for sl in csl:
                            nc.tensor.matmul(z1n[th][:, sl], wc23b[:, :],
                                             xb[th][par][:, sl],
                                             start=True, stop=False)
                    for th in range(NTH):
                        for sl in csl:
                            nc.tensor.matmul(z1n[th][:, sl], ch_t[:, :],
                                             h2s[th][:, sl],
                                             start=False, stop=True)
                    z1s = z1n
                elif s == 2:
                    # q = h2_0 + 2*(h2_1 + h2_2) on DVE (off-critical-path)
                    for th in range(NTH):
                        e = qpool.tile([128, W], F16, tag=f"e{th}", name="e")
                        nc.vector.tensor_add(e[:, :], h2keep[th][1][:, :],
                                             h2keep[th][2][:, :])
                        g = qpool.tile([128, W], F16, tag=f"g{th}", name="g")
                        nc.vector.tensor_scalar_mul(g[:, :], e[:, :], 2.0)
                        q = qpool.tile([128, W], F16, tag=f"q{th}", name="q")
                        nc.vector.tensor_add(q[:, :], g[:, :],
                                             h2keep[th][0][:, :])
                        qs[th] = q
                    z1n = [zalloc(f"z1_{th}") for th in range(NTH)]
                    for th in range(NTH):
                        for sl in csl:
                            nc.tensor.matmul(z1n[th][:, sl], wc4[:, :],
                                             xb[th][par][:, sl],
                                             start=True, stop=False)
                    for th in range(NTH):
                        for sl in csl:
                            nc.tensor.matmul(z1n[th][:, sl], cf_t[:, :],
                                             h2s[th][:, sl],
                                             start=False, stop=True)
                    z1s = z1n
                else:
                    # boundary: finish hoisted stage-0 group with the direct
                    # C6 @ h2_3 term (no DVE on the ACT chain); hs = q + h2_3
                    # feeds the y increment and next step's stage-1 C6 term.
                    if not last:
                        for th in range(NTH):
                            for sl in csl:
                                nc.tensor.matmul(z1n[th][:, sl], c6_t[:, :],
                                                 h2s[th][:, sl],
                                                 start=False, stop=True)
                        z1s = z1n
                    hss = []
                    for th in range(NTH):
                        hs = qpool.tile([128, W], F16, tag=f"hs{th}",
                                        name="hs")
                        nc.vector.tensor_add(hs[:, :], qs[th][:, :],
                                             h2s[th][:, :])
                        hss.append(hs)
                    hs_prev = hss
                    ypsum = [yalloc(f"yp{th}") for th in range(NTH)]
                    for th in range(NTH):
                        for sl in csl:
                            nc.tensor.matmul(ypsum[th][0:3, sl], w36[:, :],
                                             hss[th][:, sl],
                                             start=True, stop=True)
                    for th in range(NTH):
                        nc.vector.tensor_add(ys[th][nxt][:, :],
                                             ypsum[th][0:3, :],
                                             ys[th][par][:, :])
                        if has_b3:
                            nc.vector.tensor_scalar_add(ys[th][nxt][:, :],
                                                        ys[th][nxt][:, :],
                                                        hb3t[:, 0:1])
                        nc.sync.dma_start(yout[:, n, th * W:(th + 1) * W],
                                          ys[th][nxt][:, :])
                    # y_new rows of the next x tile are written early in the
                    # next step (deferred so stage-1's m1 DVE op goes first)
                    if not last:
                        tc_pending = [(xb[th][nxt], ys[th][nxt])
                                      for th in range(NTH)]


def build_program(B_core, T, NTH, has_b3=False, debug=False,
                  enable_asserts=False):
    nc = bacc.Bacc("TRN2", target_bir_lowering=False, debug=debug,
                   enable_asserts=enable_asserts, num_devices=1)
    shapes = {
        "xinit": ((KX, B_core), F16),
        "uT": ((T * 4, B_core), F16),
        "y0T": ((3, B_core), F32),
        "Wc1p": ((KX, 128), F16), "Wc1c": ((KX, 128), F16),
        "Wc23n": ((KX, 128), F16), "Wc23o": ((KX, 128), F16),
        "Wc23b": ((KX, 128), F16), "Wc4": ((KX, 128), F16),
        "Ch": ((128, 128), F16), "Cf": ((128, 128), F16),
        "C6": ((128, 128), F16), "W2T": ((128, 128), F16),
        "W36": ((128, 3), F16),
        "wtt": ((128, T), F32), "b2": ((128, 1), F32),
    }
    if has_b3:
        shapes["hb3"] = ((3, 1), F32)
    aps = {}
    for name, (shp, dt) in shapes.items():
        aps[name] = nc.dram_tensor(name, list(shp), dt,
                                   kind="ExternalInput").ap()
    aps["yout"] = nc.dram_tensor("yout", [3, T - 1, B_core], F32,
                                 kind="ExternalOutput").ap()
    with tile.TileContext(nc) as tc:
        build_tile_body(tc, aps, B_core, T, NTH, has_b3)
    nc.compile()
    return nc


def make_in_maps(y0, t, u, p, W1, b1, W2, b2, W3, b3, n_cores, B_core, T,
                 has_b3):
    f32 = np.float32
    f16 = np.float16
    y0 = np.asarray(y0, f32); u = np.asarray(u, f32); p = np.asarray(p, f32)
    consts = prepare_consts(W1, b1, W2, b2, W3, b3, t)
    if not has_b3:
        consts.pop("hb3")
    in_maps = []
    for i in range(n_cores):
        sl = slice(i * B_core, (i + 1) * B_core)
        xinit = np.zeros((KX, B_core), f16)
        xinit[0:3] = y0[sl].T.astype(f16)
        xinit[3] = 1.0
        xinit[4:9] = p[sl].T.astype(f16)
        xinit[YN:YN + 3] = y0[sl].T.astype(f16)
        uT = np.ascontiguousarray(
            u[sl].transpose(1, 2, 0).reshape(T * 4, B_core).astype(f16))
        y0T = np.ascontiguousarray(y0[sl].T)
        m = {"xinit": xinit, "uT": uT, "y0T": y0T}
        m.update(consts)
        in_maps.append(m)
    return in_maps


_PROGRAM_CACHE = {}


def _get_program(B_core, T, NTH, has_b3):
    key = (B_core, T, NTH, has_b3)
    if key not in _PROGRAM_CACHE:
        _PROGRAM_CACHE[key] = build_program(B_core, T, NTH, has_b3)
    return _PROGRAM_CACHE[key]


def run_on_cores(inputs, n_cores=N_CORES, NTH=2, trace=False):
    y0 = np.asarray(inputs["y0"], np.float32)
    B = y0.shape[0]
    T = np.asarray(inputs["t"]).shape[0]
    B_core = B // n_cores
    has_b3 = bool(np.any(np.asarray(inputs["b3"]) != 0))
    nc = _get_program(B_core, T, NTH, has_b3)
    in_maps = make_in_maps(
        inputs["y0"], inputs["t"], inputs["u"], inputs["p"],
        inputs["W1"], inputs["b1"], inputs["W2"], inputs["b2"],
        inputs["W3"], inputs["b3"], n_cores, B_core, T, has_b3)
    res = run_bass_kernel_spmd(nc, in_maps, list(range(n_cores)), trace=trace)
    out = np.empty((B, T, 3), np.float32)
    for i in range(n_cores):
        sl = slice(i * B_core, (i + 1) * B_core)
        yo = np.asarray(res.results[i]["yout"])        # (3, T-1, B_core)
        out[sl, 1:, :] = yo.transpose(2, 1, 0)
        out[sl, 0, :] = y0[sl]
    return out, res


def kernel(y0, t, u, p, W1, b1, W2, b2, W3, b3):
    out, _ = run_on_cores(
        dict(y0=y0, t=t, u=u, p=p, W1=W1, b1=b1, W2=W2, b2=b2,
             W3=W3, b3=b3),
        n_cores=N_CORES, NTH=2, trace=False)
    return out
